# revision 22
# baseline (speedup 1.0000x reference)
"""Dense transformer (B=4,T=1024,C=1024,H=16,L=8) on 8 TRN2 NeuronCores.

Sharding: DP over batch (4) x sequence block-cyclic (2). Core c = 2b+s handles
batch b, token blocks {j : j%2==s} (128 tokens each, 512 tokens/core). Only
attention needs cross-core data: per layer, K/V are exchanged between pair
cores via two 2-rank AllGathers (K first, then V), overlapped with the Q
projection and the attention pass over local K/V.

Layouts: activations live transposed [C, T_local] (channels on partitions) so
the whole layer chain runs with zero transposes. LayerNorm gains/biases/means
are folded into the weights on the host (rank-1 correction); the per-token
rstd is applied POST-matmul at eviction time (valid since the scale is
per-token and matmuls contract channels), so matmuls never wait on LN stats.
Weight-stationary matmuls run f32r (full PE speed); attention and the widest
streams run bf16. The fp32 residual is carried as f32r.
"""
import sys
import os
import numpy as np
import ml_dtypes

sys.path.insert(0, '/opt/trn_rl_repo')

import concourse.bass as bass  # noqa: F401
import concourse.tile as tile
from concourse import bacc, mybir
from concourse.bass_utils import run_bass_kernel_spmd

F = mybir.ActivationFunctionType
ALU = mybir.AluOpType
dt = mybir.dt
AF32 = np.float32
ABF16 = ml_dtypes.bfloat16

B, T, C, H, L = 4, 1024, 1024, 16, 8
DIN, DOUT, DH = 128, 256, 64
TL = 512
NB = 4
NCH = C // 128
DFF = 4 * C
EPS = 1e-5
KW = NCH * TL
VW = NB * (H * 65)

_CACHE = {}
L_RUN = int(os.environ.get('KLAYERS', str(L)))


def _build(use_bias):
    """use_bias: dict of bools for matrices whose folded bias is nonzero."""
    nc = bacc.Bacc("TRN2", target_bir_lowering=False, debug=False, num_devices=8)

    def par(name, shape, dtp):
        return nc.declare_dram_parameter(name, list(shape), dtp, isOutput=False)

    xloc = par('xloc', [128, TL], dt.bfloat16)
    xshift = par('xshift', [128, TL], dt.float32)
    emb_w = par('emb_w', [128, C], dt.bfloat16)
    emb_b = par('emb_b', [128, NCH], dt.float32)
    wq = par('wq', [L, C, C], dt.bfloat16)
    wk = par('wk', [L, C, C], dt.bfloat16)
    wv = par('wv', [L, C, C], dt.bfloat16)
    bq = par('bq', [L, 128, NCH], dt.float32)
    bk = par('bk', [L, 128, NCH], dt.float32)
    bv = par('bv', [L, 128, NCH], dt.float32)
    wo = par('wo', [L, C, C], dt.bfloat16)
    bo = par('bo', [L, 128, NCH], dt.float32)
    w1 = par('w1', [L, C, DFF], dt.bfloat16)
    b1 = par('b1', [L, 128, 32], dt.float32)
    w2 = par('w2', [L, DFF, C], dt.bfloat16)
    b2 = par('b2', [L, 128, NCH], dt.float32)
    pw = par('pw', [C, DOUT], dt.bfloat16)
    pb = par('pb', [128, 2], dt.float32)
    dw1 = par('dw1', [DOUT, DOUT], dt.bfloat16)
    db1 = par('db1', [128, 2], dt.float32)
    dw2 = par('dw2', [DOUT, DIN], dt.bfloat16)
    db2 = par('db2', [128, 1], dt.float32)
    mask_tri = par('mask_tri', [128, 128], dt.bfloat16)
    mask_p0 = par('mask_p0', [128, 128], dt.bfloat16)
    selp = par('selp', [128, 2], dt.float32)
    out_p = nc.declare_dram_parameter('out', [TL, 128], dt.float32, isOutput=True)

    with tile.TileContext(nc, num_cores=8) as tc:
        with tc.tile_pool(name='persist', bufs=1) as pp, \
             tc.tile_pool(name='sbwork', bufs=1) as wkp, \
             tc.tile_pool(name='wslab', bufs=1) as wsp, \
             tc.tile_pool(name='small', bufs=1) as smp, \
             tc.tile_pool(name='dram', bufs=2, space='DRAM') as drp:

            hT = pp.tile([128, NCH, TL], dt.float32r, name='hT')
            QT = pp.tile([128, NCH, TL], dt.bfloat16, name='QT')
            KTl = pp.tile([128, NCH, TL], dt.bfloat16, name='KTl')
            Vl = pp.tile([128, NB, H * 65], dt.bfloat16, name='Vl')
            peer = pp.tile([128, KW + VW], dt.bfloat16, name='peer')
            KTp = peer[:, 0:KW].rearrange("p (c t) -> p c t", c=NCH)
            Vp = peer[:, KW:KW + VW].rearrange("p (j e) -> p j e", j=NB)
            y_all = pp.tile([128, NCH, TL], dt.bfloat16, name='y_all')
            m_act = pp.tile([128, 32, TL], dt.bfloat16, name='m_act')
            mtri = pp.tile([128, 128], dt.bfloat16, name='mtri')
            mp0 = pp.tile([128, 128], dt.bfloat16, name='mp0')
            selc = pp.tile([128, 2], dt.float32, name='selc')
            ones_col = pp.tile([128, 1], dt.float32r, name='ones_col')
            ones_f32 = pp.tile([128, 1], dt.float32, name='ones_f32')
            eps_t = pp.tile([1, 1], dt.float32, name='eps_t')
            xl_sb = pp.tile([128, TL], dt.bfloat16, name='xl_sb')
            xsh_sb = pp.tile([128, TL], dt.float32, name='xsh_sb')
            featsT = pp.tile([128, 2, TL], dt.bfloat16, name='featsT')
            zT = pp.tile([128, 2, TL], dt.float32r, name='zT')
            out_sb = pp.tile([128, TL], dt.float32, name='out_sb')

            nc.sync.dma_start(mtri[:], mask_tri[:])
            nc.sync.dma_start(mp0[:], mask_p0[:])
            nc.sync.dma_start(selc[:], selp[:])
            nc.sync.dma_start(xl_sb[:], xloc[:])
            nc.sync.dma_start(xsh_sb[:], xshift[:])
            nc.vector.memset(ones_f32[:], 1.0)
            nc.vector.tensor_copy(ones_col[:], ones_f32[:])
            nc.vector.memset(eps_t[:], EPS)
            va = Vl[:].rearrange("p j (h e) -> p j h e", e=65)
            nc.vector.memset(va[:, :, :, 64:65], 1.0)

            psA = None

            def mk_htb():
                return wkp.tile([128, NCH, TL], dt.bfloat16, tag='hTb', bufs=1,
                                name='hTb')

            def ln_scale(src, nch):
                """Per-token rstd of src [128, nch, TL] -> bcast [128, TL] f32."""
                ps_su = psA.tile([1, TL], dt.float32, tag='ps', bufs=8, name='ps_su')
                for c in range(nch):
                    nc.tensor.matmul(ps_su[:], ones_col[:], src[:, c, :],
                                     start=(c == 0), stop=(c == nch - 1))
                mu = smp.tile([1, TL], dt.float32, tag='lnmu', bufs=1, name='mu')
                nc.scalar.activation(mu[:], ps_su[:], F.Copy, scale=1.0 / (nch * 128))
                ps_sq = psA.tile([1, TL], dt.float32, tag='ps', bufs=8, name='ps_sq')
                for c in range(nch):
                    sq = wkp.tile([128, TL], dt.float32r, tag='sq', bufs=2, name='sq')
                    nc.scalar.activation(sq[:], src[:, c, :], F.Square)
                    nc.tensor.matmul(ps_sq[:], ones_col[:], sq[:],
                                     start=(c == 0), stop=(c == nch - 1))
                var = smp.tile([1, TL], dt.float32, tag='lnvar', bufs=1, name='var')
                nc.scalar.activation(var[:], ps_sq[:], F.Copy, scale=1.0 / (nch * 128))
                mu2 = smp.tile([1, TL], dt.float32, tag='lnmu2', bufs=1, name='mu2')
                nc.vector.tensor_mul(mu2[:], mu[:], mu[:])
                nc.vector.tensor_sub(var[:], var[:], mu2[:])
                nc.scalar.activation(var[:], var[:], F.Sqrt, bias=eps_t[0:1, 0:1])
                rstd = smp.tile([1, TL], dt.float32, tag='lnrstd', bufs=1, name='rstd')
                rscr = smp.tile([1, TL], dt.float32, tag='lnrscr', bufs=1, name='rscr')
                nc.vector.reciprocal_approx_accurate(rstd[:], var[:], rscr[:])
                sb = wkp.tile([128, TL], dt.float32, tag='lnsb', bufs=2, name='sb')
                nc.gpsimd.partition_broadcast(sb[:], rstd[0:1, :])
                return sb

            def matphase(src, w_ap, l, nch_in, nch_out, out_cb, wtag, wdt):
                """out[co] = sum_ci w[ci,co].T @ src[:,ci,:]; evict out_cb."""
                pss = [psA.tile([128, TL], dt.float32, tag='ps', bufs=8,
                                name=f'pp{co}') for co in range(nch_out)]
                for ci in range(nch_in):
                    slab = wsp.tile([128, nch_out * 128], wdt, tag=wtag,
                                    bufs=(6 if wtag == 'wslab' else 8), name='slab')
                    src_w = w_ap[l, ci * 128:(ci + 1) * 128, :] if l is not None \
                        else w_ap[ci * 128:(ci + 1) * 128, :]
                    nc.sync.dma_start(slab[:], src_w)
                    for co in range(nch_out):
                        nc.tensor.matmul(pss[co][:], slab[:, co * 128:(co + 1) * 128],
                                         src[:, ci, :], start=(ci == 0),
                                         stop=(ci == nch_in - 1))
                for co in range(nch_out):
                    out_cb(co, pss[co])

            # ---------------- embed ----------------
            with tc.tile_pool(name='psE', bufs=1, space='PSUM') as psA:
                embs = wsp.tile([128, C], dt.bfloat16, tag='wslabb', bufs=8,
                                name='embs')
                nc.sync.dma_start(embs[:], emb_w[:])
                ebias = smp.tile([128, NCH], dt.float32, tag='bias8', bufs=2,
                                 name='ebias')
                nc.sync.dma_start(ebias[:], emb_b[:])
                for co in range(NCH):
                    ps = psA.tile([128, TL], dt.float32, tag='ps', bufs=8,
                                  name=f'pe{co}')
                    nc.tensor.matmul(ps[:], embs[:, co * 128:(co + 1) * 128],
                                     xl_sb[:], start=True, stop=True)
                    nc.scalar.activation(hT[:, co, :], ps[:], F.Identity,
                                         bias=ebias[:, co:co + 1])
                hTb = mk_htb()
                for co in range(NCH):
                    nc.vector.tensor_copy(hTb[:, co, :], hT[:, co, :])

            # ---------------- layers ----------------
            for li in range(L_RUN):
                with tc.tile_pool(name=f'psA{li}', bufs=1, space='PSUM') as psA:
                    sb1 = ln_scale(hT, NCH)

                    kb = smp.tile([128, NCH], dt.float32, tag='bias8', bufs=2,
                                  name='kb')
                    if use_bias['qkv']:
                        nc.sync.dma_start(kb[:], bk[li])

                    def evict_k(co, ps):
                        nc.vector.tensor_mul(KTl[:, co, :], ps[:], sb1[:])
                        if use_bias['qkv']:
                            nc.vector.tensor_scalar_add(KTl[:, co, :], KTl[:, co, :],
                                                        kb[:, co:co + 1])
                    matphase(hTb, wk, li, NCH, NCH, evict_k, 'wslabb', dt.bfloat16)

                    inbK = drp.tile([128, KW], dt.bfloat16, tag='inbK', bufs=2,
                                    name='inbK')
                    outbK = drp.tile([256, KW], dt.bfloat16, tag='outbK', bufs=2,
                                     name='outbK')
                    kfl = KTl[:].rearrange("p c t -> p (c t)")
                    for ch in range(4):
                        cs = slice(ch * (KW // 4), (ch + 1) * (KW // 4))
                        nc.sync.dma_start(inbK[:, cs], kfl[:, cs])
                    nc.gpsimd.collective_compute(
                        "AllGather", ALU.bypass,
                        replica_groups=[[0, 1], [2, 3], [4, 5], [6, 7]],
                        ins=[inbK.opt()], outs=[outbK.opt()])

                    xs = wkp.tile([128, NCH, TL], dt.bfloat16, tag='xs', bufs=1,
                                  name='xs')
                    for c in range(NCH):
                        nc.vector.tensor_mul(xs[:, c, :], hT[:, c, :], sb1[:])

                    psv = [psA.tile([128, TL], dt.float32, tag='ps', bufs=8,
                                    name=f'pv{i}') for i in range(8)]
                    for ci in range(NCH):
                        slab = wsp.tile([128, C], dt.bfloat16, tag='wslabb', bufs=8,
                                        name='vslab')
                        nc.sync.dma_start(slab[:], wv[li, ci * 128:(ci + 1) * 128, :])
                        for tb in range(NB):
                            for dvh in range(2):
                                nc.tensor.matmul(
                                    psv[tb * 2 + dvh][:],
                                    xs[:, ci, tb * 128:(tb + 1) * 128],
                                    slab[:, dvh * 512:(dvh + 1) * 512],
                                    start=(ci == 0), stop=(ci == NCH - 1))
                    for tb in range(NB):
                        for dvh in range(2):
                            dst = va[:, tb, dvh * 8:(dvh + 1) * 8, 0:64]
                            src = psv[tb * 2 + dvh][:].rearrange(
                                "p (h e) -> p h e", e=64)
                            nc.scalar.activation(dst, src, F.Copy)

                    inbV = drp.tile([128, VW], dt.bfloat16, tag='inbV', bufs=2,
                                    name='inbV')
                    outbV = drp.tile([256, VW], dt.bfloat16, tag='outbV', bufs=2,
                                     name='outbV')
                    vfl = Vl[:].rearrange("p j e -> p (j e)")
                    for ch in range(4):
                        cs = slice(ch * (VW // 4), (ch + 1) * (VW // 4))
                        nc.sync.dma_start(inbV[:, cs], vfl[:, cs])
                    nc.gpsimd.collective_compute(
                        "AllGather", ALU.bypass,
                        replica_groups=[[0, 1], [2, 3], [4, 5], [6, 7]],
                        ins=[inbV.opt()], outs=[outbV.opt()])

                    def do_select(outb_b, base, wdth):
                        W4 = wdth // 4
                        for ch in range(4):
                            c0 = ch * W4
                            h1t = wkp.tile([128, W4], dt.bfloat16, tag='h1', bufs=2,
                                           name='h1t')
                            nc.sync.dma_start(peer[:, base + c0:base + c0 + W4],
                                              outb_b[0:128, c0:c0 + W4])
                            nc.sync.dma_start(h1t[:], outb_b[128:256, c0:c0 + W4])
                            nc.vector.tensor_scalar_mul(
                                peer[:, base + c0:base + c0 + W4],
                                peer[:, base + c0:base + c0 + W4], selc[:, 0:1])
                            nc.vector.scalar_tensor_tensor(
                                peer[:, base + c0:base + c0 + W4], h1t[:],
                                selc[:, 1:2], peer[:, base + c0:base + c0 + W4],
                                ALU.mult, ALU.add)
                    do_select(outbK, 0, KW)

                    qb = smp.tile([128, NCH], dt.float32, tag='bias8', bufs=2,
                                  name='qb')
                    if use_bias['qkv']:
                        nc.sync.dma_start(qb[:], bq[li])

                    def evict_q(co, ps):
                        nc.vector.tensor_mul(QT[:, co, :], ps[:], sb1[:])
                        if use_bias['qkv']:
                            nc.vector.tensor_scalar_add(QT[:, co, :], QT[:, co, :],
                                                        qb[:, co:co + 1])
                    matphase(hTb, wq, li, NCH, NCH, evict_q, 'wslabb', dt.bfloat16)

                    do_select(outbV, KW, VW)

                with tc.tile_pool(name=f'psB{li}', bufs=1, space='PSUM') as psB:
                    vbl = smp.tile([128, NCH], dt.float32, tag='bias8v', bufs=2,
                                   name='vbl')
                    if use_bias['v']:
                        nc.sync.dma_start(vbl[:], bv[li])
                    for cp in range(8):
                        psy = psB.tile([65, 2, TL], dt.float32, tag='psy', bufs=2,
                                       name='psy')
                        first = True
                        for src_i, KTs, Vs in ((0, KTl, Vl), (1, KTp, Vp)):
                            vaa = Vs.rearrange("p j (h e) -> p j h e", e=65)
                            for j in range(NB):
                                qs = 128 * j
                                qn = TL - qs
                                pssc = psB.tile([128, 2, TL], dt.float32, tag='pssc',
                                                bufs=2, name='pssc')
                                for hp in range(2):
                                    nc.tensor.matmul(
                                        pssc[:, hp, 0:qn],
                                        KTs[hp * 64:(hp + 1) * 64, cp,
                                            j * 128:(j + 1) * 128],
                                        QT[hp * 64:(hp + 1) * 64, cp, qs:TL],
                                        start=True, stop=True)
                                et = wkp.tile([128, 2, qn], dt.bfloat16, tag='et',
                                              bufs=6, name='et')
                                nc.scalar.activation(et[:], pssc[:, :, 0:qn], F.Exp)
                                msk = mtri if src_i == 0 else mp0
                                for hp in range(2):
                                    nc.vector.tensor_mul(et[:, hp, 0:128],
                                                         et[:, hp, 0:128], msk[:])
                                for hp in range(2):
                                    h = 2 * cp + hp
                                    nc.tensor.matmul(
                                        psy[:, hp, qs:TL],
                                        vaa[:, j, h, :], et[:, hp, :],
                                        start=first,
                                        stop=(src_i == 1 and j == NB - 1))
                                first = False
                        ysb = wkp.tile([64, 2, TL], dt.bfloat16, tag='ysb', bufs=3,
                                       name='ysb')
                        nc.scalar.activation(ysb[:], psy[0:64, :, :], F.Copy)
                        rr = smp.tile([1, 2, TL], dt.float32, tag='rr', bufs=1,
                                      name='rr')
                        rrd = smp.tile([1, 2, TL], dt.float32, tag='rrd', bufs=1,
                                       name='rrd')
                        rrs = smp.tile([1, 2, TL], dt.float32, tag='rrs', bufs=1,
                                       name='rrs')
                        nc.vector.tensor_copy(rrd[:], psy[64:65, :, :])
                        nc.vector.reciprocal_approx_accurate(rr[:], rrd[:], rrs[:])
                        rb = wkp.tile([64, 2, TL], dt.float32, tag='rb', bufs=1,
                                      name='rb')
                        nc.gpsimd.partition_broadcast(rb[:], rr[0:1, :, :])
                        for hp in range(2):
                            nc.vector.tensor_mul(y_all[hp * 64:(hp + 1) * 64, cp, :],
                                                 ysb[:, hp, :], rb[:, hp, :])
                            if use_bias['v']:
                                nc.vector.tensor_scalar_add(
                                    y_all[hp * 64:(hp + 1) * 64, cp, :],
                                    y_all[hp * 64:(hp + 1) * 64, cp, :],
                                    vbl[hp * 64:(hp + 1) * 64, cp:cp + 1])

                with tc.tile_pool(name=f'psC{li}', bufs=1, space='PSUM') as psA:
                    obias = smp.tile([128, NCH], dt.float32, tag='bias8o', bufs=2,
                                     name='obias')
                    nc.sync.dma_start(obias[:], bo[li])

                    def evict_proj(co, ps):
                        nc.vector.scalar_tensor_tensor(
                            hT[:, co, :], ps[:], obias[:, co:co + 1], hT[:, co, :],
                            ALU.add, ALU.add)
                    matphase(y_all, wo, li, NCH, NCH, evict_proj,
                             'wslabb', dt.bfloat16)
                    hTb = mk_htb()
                    for co in range(NCH):
                        nc.vector.tensor_copy(hTb[:, co, :], hT[:, co, :])

                    sb2 = ln_scale(hT, NCH)
                    b1s = smp.tile([128, 32], dt.float32, tag='b1s', bufs=2,
                                   name='b1s')
                    if use_bias['fc1']:
                        nc.sync.dma_start(b1s[:], b1[li])
                    for fog in range(4):
                        psf = [psA.tile([128, TL], dt.float32, tag='ps', bufs=8,
                                        name=f'pf{i}') for i in range(8)]
                        for ci in range(NCH):
                            slab = wsp.tile([128, C], dt.bfloat16, tag='wslabb',
                                            bufs=8, name='f1slab')
                            nc.sync.dma_start(
                                slab[:], w1[li, ci * 128:(ci + 1) * 128,
                                            fog * 1024:(fog + 1) * 1024])
                            for fo in range(8):
                                nc.tensor.matmul(psf[fo][:],
                                                 slab[:, fo * 128:(fo + 1) * 128],
                                                 hTb[:, ci, :], start=(ci == 0),
                                                 stop=(ci == NCH - 1))
                        for fo in range(8):
                            fi = fog * 8 + fo
                            mtmp = wkp.tile([128, TL], dt.bfloat16, tag='mtmp',
                                            bufs=3, name='mtmp')
                            nc.vector.tensor_mul(mtmp[:], psf[fo][:], sb2[:])
                            if use_bias['fc1']:
                                nc.vector.tensor_scalar_add(mtmp[:], mtmp[:],
                                                            b1s[:, fi:fi + 1])
                            nc.scalar.activation(m_act[:, fi, :], mtmp[:], F.Gelu)
                    b2s = smp.tile([128, NCH], dt.float32, tag='bias8', bufs=2,
                                   name='b2s')
                    nc.sync.dma_start(b2s[:], b2[li])
                    psm = [psA.tile([128, TL], dt.float32, tag='ps', bufs=8,
                                    name=f'pm{i}') for i in range(8)]
                    for fi in range(32):
                        slab = wsp.tile([128, C], dt.bfloat16, tag='wslabb', bufs=8,
                                        name='f2slab')
                        nc.sync.dma_start(slab[:], w2[li, fi * 128:(fi + 1) * 128, :])
                        for co in range(NCH):
                            nc.tensor.matmul(psm[co][:],
                                             slab[:, co * 128:(co + 1) * 128],
                                             m_act[:, fi, :], start=(fi == 0),
                                             stop=(fi == 31))
                    for co in range(NCH):
                        nc.vector.scalar_tensor_tensor(
                            hT[:, co, :], psm[co][:], b2s[:, co:co + 1], hT[:, co, :],
                            ALU.add, ALU.add)
                    hTb = mk_htb()
                    for co in range(NCH):
                        nc.vector.tensor_copy(hTb[:, co, :], hT[:, co, :])

            # ---------------- head ----------------
            with tc.tile_pool(name='psH', bufs=1, space='PSUM') as psA:
                sbf = ln_scale(hT, NCH)
                pbias = smp.tile([128, 2], dt.float32, tag='bias2', bufs=2,
                                 name='pbias')
                if use_bias['pw']:
                    nc.sync.dma_start(pbias[:], pb[:])

                def evict_pw(co, ps):
                    nc.vector.tensor_mul(featsT[:, co, :], ps[:], sbf[:])
                    if use_bias['pw']:
                        nc.vector.tensor_scalar_add(featsT[:, co, :],
                                                    featsT[:, co, :],
                                                    pbias[:, co:co + 1])
                matphase(hTb, pw, None, NCH, 2, evict_pw, 'wslabb', dt.bfloat16)

                d1b = smp.tile([128, 2], dt.float32, tag='bias2', bufs=2, name='d1b')
                if use_bias['dw1']:
                    nc.sync.dma_start(d1b[:], db1[:])

                def evict_d1(co, ps):
                    nc.scalar.activation(zT[:, co, :], ps[:], F.Tanh,
                                         bias=(d1b[:, co:co + 1]
                                               if use_bias['dw1'] else 0.0))
                matphase(featsT, dw1, None, 2, 2, evict_d1, 'wslabb', dt.bfloat16)

                sbz = ln_scale(zT, 2)
                zb = wkp.tile([128, 2, TL], dt.bfloat16, tag='zb', bufs=1, name='zb')
                for co in range(2):
                    nc.vector.tensor_copy(zb[:, co, :], zT[:, co, :])
                d2b = smp.tile([128, 1], dt.float32, tag='bias2', bufs=2, name='d2b')
                if use_bias['dw2']:
                    nc.sync.dma_start(d2b[:], db2[:])

                def evict_out(co, ps):
                    ptmp = wkp.tile([128, TL], dt.float32, tag='ptmp', bufs=1,
                                    name='ptmp')
                    nc.vector.tensor_mul(ptmp[:], ps[:], sbz[:])
                    if use_bias['dw2']:
                        nc.vector.scalar_tensor_tensor(out_sb[:], ptmp[:],
                                                       d2b[:, 0:1], xsh_sb[:],
                                                       ALU.add, ALU.subtract)
                    else:
                        nc.vector.tensor_sub(out_sb[:], ptmp[:], xsh_sb[:])
                matphase(zb, dw2, None, 2, 1, evict_out, 'wslabb', dt.bfloat16)
                nc.sync.dma_start(out_p.rearrange("t d -> d t"), out_sb[:])

    nc.compile()
    return nc


def _fold(g, b, W, bias, scl=1.0):
    """LN(x;g,b) @ W + bias == (x @ W'')*rstd + v with the mean folded in."""
    g = np.asarray(g, np.float64)
    W = np.asarray(W, np.float64)
    u = g @ W
    Wf = (g[:, None] * W - u[None, :] / W.shape[0]) * scl
    v = (np.asarray(b, np.float64) @ W + np.asarray(bias, np.float64)) * scl
    return Wf.astype(AF32), v.astype(AF32)


def _r8(v):
    return np.ascontiguousarray(np.asarray(v, AF32).reshape(-1, 128).T)


def kernel(**inputs):
    x = np.asarray(inputs['tokens'], AF32).reshape(B, T, DIN)
    ln1_g, ln1_b = inputs['ln1_g'], inputs['ln1_b']
    ln2_g, ln2_b = inputs['ln2_g'], inputs['ln2_b']
    scale = 1.0 / np.sqrt(C // H)

    wq = np.empty((L, C, C), ABF16); bqh = np.empty((L, 128, NCH), AF32)
    wk = np.empty((L, C, C), ABF16); bkh = np.empty((L, 128, NCH), AF32)
    wv = np.empty((L, C, C), ABF16); bvh = np.empty((L, 128, NCH), AF32)
    woh = np.empty((L, C, C), ABF16); boh = np.empty((L, 128, NCH), AF32)
    w1h = np.empty((L, C, DFF), ABF16); b1h = np.empty((L, 128, 32), AF32)
    w2h = np.empty((L, DFF, C), ABF16); b2h = np.empty((L, 128, NCH), AF32)
    for l in range(L):
        Wf, v = _fold(ln1_g[l], ln1_b[l], inputs['Wq'][l], inputs['bq'][l], scale)
        wq[l], bqh[l] = Wf, _r8(v)
        Wf, v = _fold(ln1_g[l], ln1_b[l], inputs['Wk'][l], inputs['bk'][l])
        wk[l], bkh[l] = Wf, _r8(v)
        Wf, v = _fold(ln1_g[l], ln1_b[l], inputs['Wv'][l], inputs['bv'][l])
        wv[l], bvh[l] = Wf, _r8(v)
        woh[l] = np.asarray(inputs['Wo'][l], AF32).astype(ABF16)
        boh[l] = _r8(inputs['bo'][l])
        Wf, v = _fold(ln2_g[l], ln2_b[l], inputs['W1'][l], inputs['b1'][l])
        w1h[l], b1h[l] = Wf, _r8(v)
        w2h[l] = np.asarray(inputs['W2'][l], AF32).astype(ABF16)
        b2h[l] = _r8(inputs['b2'][l])

    pwf, pv = _fold(inputs['lnf_g'], inputs['lnf_b'], inputs['proj_w'],
                    inputs['proj_b'])
    dw2f, d2v = _fold(inputs['dec_ln_g'], inputs['dec_ln_b'], inputs['dec_w2'],
                      inputs['dec_b2'])
    d1v = np.asarray(inputs['dec_b1'], AF32)

    use_bias = {
        'qkv': bool(np.abs(bqh).max() > 0 or np.abs(bkh).max() > 0),
        'v': bool(np.abs(bvh).max() > 0),
        'fc1': bool(np.abs(b1h).max() > 0),
        'pw': bool(np.abs(pv).max() > 0),
        'dw1': bool(np.abs(d1v).max() > 0),
        'dw2': bool(np.abs(d2v).max() > 0),
    }
    key = tuple(sorted(use_bias.items()))
    if key not in _CACHE:
        _CACHE[key] = _build(use_bias)
    nc = _CACHE[key]

    tri = np.tril(np.ones((128, 128), AF32)).T.astype(ABF16)
    shared = dict(
        emb_w=np.asarray(inputs['tok_emb_w'], AF32).astype(ABF16),
        emb_b=_r8(inputs['tok_emb_b']),
        wq=wq, wk=wk, wv=wv, bq=bqh, bk=bkh, bv=bvh, wo=woh, bo=boh,
        w1=w1h, b1=b1h, w2=w2h, b2=b2h,
        pw=pwf.astype(ABF16), pb=_r8(pv),
        dw1=np.asarray(inputs['dec_w1'], AF32).astype(ABF16), db1=_r8(d1v),
        dw2=dw2f.astype(ABF16), db2=np.ascontiguousarray(d2v.reshape(1, 128).T),
        mask_tri=tri,
    )

    in_maps = []
    for c in range(8):
        b_, s = c // 2, c % 2
        tloc = np.concatenate([np.arange(128 * (2 * j + s), 128 * (2 * j + s) + 128)
                               for j in range(NB)])
        tnext = np.minimum(tloc + 1, T - 1)
        im = dict(shared)
        im['xloc'] = np.ascontiguousarray(x[b_][tloc].T).astype(ABF16)
        im['xshift'] = np.ascontiguousarray(x[b_][tnext].T)
        im['mask_p0'] = np.full((128, 128), float(s), AF32).astype(ABF16)
        im['selp'] = np.tile(np.array([[float(s), 1.0 - s]], AF32), (128, 1))
        in_maps.append(im)

    res = run_bass_kernel_spmd(nc, in_maps, list(range(8)))
    out = np.empty((B, T - 1, DIN), AF32)
    for c in range(8):
        b_, s = c // 2, c % 2
        o = res.results[c]['out']
        for j in range(NB):
            g = 2 * j + s
            t0, t1 = 128 * g, min(128 * g + 128, T - 1)
            out[b_, t0:t1] = o[128 * j:128 * j + (t1 - t0)]
    return out
